# revision 14
# baseline (speedup 1.0000x reference)
"""PolyMatchingLoss Trainium2 kernel.

Reference computation (B=128, P=1024, C=2):
    dis[b, i] = mean_j sum_c smooth_l1(pred[b,j,c] - gt[b,(i+j)%P,c])
    out = mean_b min_i dis[b, i]

Strategy (v2): two concurrent per-core lanes over the 16 local batches.

  Lane D (DVE, ND batches): one fused custom DVE instruction per
    (b, shift-block) computes 2*smooth_l1(W - P) elementwise over a
    [128 shifts, 2048 (j,c)] tile and sum-reduces along the free axis
    into a [128,1] accumulator column.  smooth_l1 via m*(2t-m), t=|d|,
    m=min(t,1).  The gt operand uses the staircase identity
    W[x, y] = gtflat2[2x+y]; pred is host-replicated to 128 partitions.

  Lane C (ACT+PE, NC batches): uses 2f(d) = d^2 - relu(|d|-1)^2.
    Layout: partition = point-in-block u, free = shift i.
    - Sum_j d^2 = (Sum p^2 + Sum g^2) - 2 corr[i]; the constant is added
      on the host, corr[i] is computed by TensorE as 8 accumulating
      matmuls per c whose stationary operand is the -2*pred block column
      and whose moving operand is the SAME gt window tile the ACT passes
      read.
    - The correction Sum_j relu(|d|-1)^2 runs on ScalarE in 3 passes
      (Abs with per-partition -pred bias; one big Relu(x-1); one big
      Square) and is reduced over partitions by TensorE with a -1s
      stationary column.
    - All matmuls of all NC batches accumulate into one stacked PSUM
      pair [8, 512]x2 via one-hot stationary columns (col = local batch
      index), so PSUM is copied out exactly once per rep.

  min over shifts + mean over batch on host (tiny).
"""

from operator import add as _operator_add

import numpy as np

from concourse import mybir
from concourse import bass, bass_utils
from concourse.tile import TileContext
import concourse.dve_ops as _dve_ops
from concourse.dve_ops import DveOp
from concourse.dve_spec import Spec, Src0, Src1, Zero, One, maxx, minn

# ---------------------------------------------------------------------------
# Workaround: this toolchain's walrus allows at most ONE sync wait per
# instruction; Tile emits 2+.  Split extras onto EventSemaphore carrier
# instructions inserted just before the offending instruction.
# ---------------------------------------------------------------------------
def _split_multi_waits(nc) -> int:
    n = 0
    for fn in nc.m.functions:
        for bb in fn.blocks:
            out = []
            for inst in bb.instructions:
                si = inst.sync_info
                if si is not None and si.on_wait and len(si.on_wait) > 1:
                    for k, w in enumerate(si.on_wait[:-1]):
                        out.append(
                            mybir.InstEventSemaphore(
                                name=f"{inst.name}_wsplit{k}",
                                opcode="EventSemaphore",
                                engine=inst.engine,
                                ins=[],
                                outs=[],
                                sync_info=mybir.SyncInfo(on_wait=[w], on_update=[]),
                            )
                        )
                        n += 1
                    si.on_wait = [si.on_wait[-1]]
                out.append(inst)
            bb.instructions = out
    return n


B = 128
PNUM = 1024
C = 2
NCORES = 8
BL = B // NCORES  # batches per core
FD = PNUM * C  # 2048 free elements per lane-D tile
WW = FD + 256 * 7  # 3840 lane-D window width
WB = 1024 + 128 * 7  # 1920 lane-C per-coordinate window width

ND = 11  # lane-D (DVE) batches per core
NC = BL - ND  # lane-C (ACT+PE) batches per core


# --------------------------------------------------------------------------
# Custom DVE op: out = m*(2t - m) with t=|in0-in1|, m=min(t,1)  (= 2*huber)
#                accum_out = sum over free axis
# --------------------------------------------------------------------------
def _huber_ref(in0, in1, s0, s1, imm2):
    dd = in0.astype(np.float32) - in1.astype(np.float32)
    tt = np.abs(dd)
    mm = np.minimum(tt, 1.0)
    bb = (mm * (2.0 * tt - mm)).astype(np.float32)
    return bb, bb.reshape(bb.shape[0], -1).sum(axis=-1, keepdims=True)


def _make_huber_op() -> DveOp:
    d = Src0 - Src1
    nd = Src1 - Src0
    t = maxx(d, nd)
    m = minn(t, One)
    v = t - m
    w = t + v
    body = m * w
    return DveOp(
        "TENSOR_HUBER2_REDUCE",
        Spec(body=body, accum=_operator_add, accum_init=Zero, reference=_huber_ref),
        subdim=False,
        uops_sha={"v3": "e8f6160a1f1db788", "v4": "8b26f7daea78cb80"},
    )


def _register_op(op: DveOp) -> None:
    if op.name in _dve_ops._SUB_OPCODE_FOR_NAME:
        return
    _dve_ops.OPS.append(op)
    _dve_ops._SUB_OPCODE_FOR_NAME[op.name] = (
        _dve_ops._CUSTOM_DVE_ROW_BASE + len(_dve_ops.OPS) - 1
    )
    _dve_ops.CUSTOM_DVE_SPECS[op.name] = op.spec
    assert _dve_ops._SUB_OPCODE_FOR_NAME[op.name] < 0x20


HUBER_OP = _make_huber_op()
_register_op(HUBER_OP)


# --------------------------------------------------------------------------
# Bass program (SPMD, one program for all 8 cores)
# --------------------------------------------------------------------------
_dt = mybir.dt
_program_cache = {}


def _build_program(reps: int = 1):
    nc = bass.Bass()
    AF = mybir.ActivationFunctionType

    NDP, NCP = max(ND, 1), max(NC, 1)
    # lane D inputs
    gtw = nc.declare_dram_parameter("gtw", [NDP, 2 * FD], _dt.float32, isOutput=False)
    prep = nc.declare_dram_parameter(
        "prep", [NDP, 128, FD], _dt.float32, isOutput=False
    )
    # lane C inputs
    gtsepb = nc.declare_dram_parameter(
        "gtsepb", [NCP, 2, 2048], _dt.bfloat16, isOutput=False
    )
    pcolc = nc.declare_dram_parameter(
        "pcolc", [NCP, 2, 128, 8], _dt.float32, isOutput=False
    )
    statp = nc.declare_dram_parameter(
        "statp", [128, NCP * 16 * 8], _dt.bfloat16, isOutput=False
    )
    stato = nc.declare_dram_parameter(
        "stato", [128, NCP * 8], _dt.bfloat16, isOutput=False
    )
    # outputs
    acc_out = nc.declare_dram_parameter(
        "acc", [128, NDP * 8], _dt.float32, isOutput=True
    )
    accc_out = nc.declare_dram_parameter("accc", [8, 1024], _dt.float32, isOutput=True)

    with TileContext(nc) as tc:
        with (
            tc.tile_pool(name="w", bufs=3) as wpool,
            tc.tile_pool(name="p", bufs=2) as ppool,
            tc.tile_pool(name="s", bufs=2) as spool,
            tc.tile_pool(name="a", bufs=1) as apool,
            tc.tile_pool(name="wb", bufs=2) as wbpool,
            tc.tile_pool(name="pc", bufs=2) as pcpool,
            tc.tile_pool(name="act", bufs=2) as actpool,
            tc.tile_pool(name="st", bufs=1) as stpool,
            tc.tile_pool(name="ac2", bufs=1) as ac2pool,
            tc.tile_pool(name="ps", bufs=1, space="PSUM") as pspool,
        ):
            acc = apool.tile([128, max(ND, 1) * 8], _dt.float32)
            nc.vector.memset(acc[:], 0.0)
            statpt = stpool.tile([128, max(NC, 1) * 16 * 8], _dt.bfloat16, tag="statpt")
            nc.sync.dma_start(out=statpt[:], in_=statp[:])
            statot = stpool.tile([128, max(NC, 1) * 8], _dt.bfloat16, tag="statot")
            nc.sync.dma_start(out=statot[:], in_=stato[:])
            onen = stpool.tile([128, 1], _dt.float32, tag="onen")
            nc.vector.memset(onen[:], -1.0)

            for _rep in range(reps):
                if NC > 0:
                    psA = pspool.tile([8, 512], _dt.float32, tag="psA")
                    psB = pspool.tile([8, 512], _dt.float32, tag="psB")

                # ---------------- lane C (ACT + PE) ----------------
                # start/stop are per PSUM bank: the first matmul into EACH
                # of psA/psB must carry start=True (clears the bank), else
                # a second execution of the NEFF accumulates onto stale
                # PSUM contents.
                nmm = 0  # per-bank matmul counter (A and B advance together)
                NMM_TOTAL = NC * 2 * 8 * 2
                for bi in range(NC):
                    wb = wbpool.tile([128, 2 * WB], _dt.bfloat16)
                    for c in (0, 1):
                        # staircase: row u = gtsepb[bi, c, u : u + WB]
                        nc.scalar.dma_start(
                            out=wb[:, c * WB : (c + 1) * WB],
                            in_=bass.AP(
                                gtsepb, (bi * 2 + c) * 2048, [[1, 128], [1, WB]]
                            ),
                        )
                    pcol = pcpool.tile([128, 16], _dt.float32)
                    for c in (0, 1):
                        nc.scalar.dma_start(
                            out=pcol[:, c * 8 : (c + 1) * 8], in_=pcolc[bi, c]
                        )
                    for c in (0, 1):
                        # corr matmuls: -2*corr into psum rows [bi]
                        for q in range(8):
                            scol = ((bi * 2 + c) * 8 + q) * 8
                            for h, ps in ((0, psA), (1, psB)):
                                nc.tensor.matmul(
                                    ps[:, :],
                                    statpt[:, scol : scol + 8],
                                    wb[:, c * WB + 128 * q + 512 * h :][:, 0:512],
                                    start=(nmm == 0),
                                    stop=(nmm == NMM_TOTAL - 1),
                                )
                            nmm += 1
                        # ACT passes
                        tt = actpool.tile([128, 8192], _dt.bfloat16, tag="tt")
                        for q in range(8):
                            nc.scalar.activation(
                                tt[:, 1024 * q : 1024 * (q + 1)],
                                wb[:, c * WB + 128 * q :][:, 0:1024],
                                AF.Abs,
                                bias=pcol[:, c * 8 + q : c * 8 + q + 1],
                            )
                        rr = actpool.tile([128, 8192], _dt.bfloat16, tag="rr")
                        nc.scalar.activation(rr[:], tt[:], AF.Relu, bias=onen[:, 0:1])
                        r2 = actpool.tile([128, 8192], _dt.bfloat16, tag="r2")
                        nc.scalar.activation(r2[:], rr[:], AF.Square)
                        # reduction matmuls: -sum_u r2 into psum rows [bi]
                        for q in range(8):
                            for h, ps in ((0, psA), (1, psB)):
                                nc.tensor.matmul(
                                    ps[:, :],
                                    statot[:, bi * 8 : bi * 8 + 8],
                                    r2[:, 1024 * q + 512 * h :][:, 0:512],
                                    start=(nmm == 0),
                                    stop=(nmm == NMM_TOTAL - 1),
                                )
                            nmm += 1

                # ---------------- lane D (DVE) ----------------
                for b in range(ND):
                    w = wpool.tile([128, WW], _dt.float32)
                    # staircase window: row x = gtflat2[b, 2x : 2x + WW]
                    nc.sync.dma_start(
                        out=w[:], in_=bass.AP(gtw, b * 2 * FD, [[2, 128], [1, WW]])
                    )
                    p = ppool.tile([128, FD], _dt.float32)
                    nc.sync.dma_start(out=p[:], in_=prep[b])
                    for qi in range(8):
                        scr = spool.tile([128, FD], _dt.float32)
                        col = b * 8 + qi
                        nc.vector._custom_dve(
                            HUBER_OP,
                            out=scr[:],
                            in0=w[:, 256 * qi : 256 * qi + FD],
                            in1=p[:],
                            accum_out=acc[:, col : col + 1],
                        )

                # psum -> sbuf -> dram for lane C (on ScalarE: the DVE is
                # the binding lane, keep it free of the copies)
                if NC > 0:
                    accc = ac2pool.tile([8, 1024], _dt.float32)
                    nc.scalar.copy(accc[:, 0:512], psA[:])
                    nc.scalar.copy(accc[:, 512:1024], psB[:])
                    nc.scalar.dma_start(out=accc_out[:], in_=accc[:])

            nc.sync.dma_start(out=acc_out[:], in_=acc[:])
    _split_multi_waits(nc)
    # Raw Bass (unlike Bacc.compile) never runs this pass; without it the
    # custom-DVE InstISA subclasses serialize with empty .instr bytes and
    # walrus fails with "ISA wrong length".
    mybir.codegen_inst_isa_subclasses(nc)
    return nc


def _get_program():
    if "nc" not in _program_cache:
        _program_cache["nc"] = _build_program()
    return _program_cache["nc"]


# --------------------------------------------------------------------------
# Host wrapper
# --------------------------------------------------------------------------
def _make_in_maps(pred: np.ndarray, gt: np.ndarray):
    pred = np.ascontiguousarray(pred, dtype=np.float32)
    gt = np.ascontiguousarray(gt, dtype=np.float32)
    in_maps = []
    for core in range(NCORES):
        sl = slice(core * BL, (core + 1) * BL)
        gtc = gt[sl]  # [BL, P, C]
        predc = pred[sl]  # [BL, P, C]
        gtdupc = np.concatenate([gtc, gtc], axis=1)  # [BL, 2P, C]
        # lane D
        NDP, NCP = max(ND, 1), max(NC, 1)
        gtdup = np.zeros((NDP, 2 * FD), np.float32)
        gtdup[:ND] = gtdupc[:ND].reshape(ND, 2 * FD)
        prepc = np.zeros((NDP, 128, FD), np.float32)
        prepc[:ND] = np.broadcast_to(predc[:ND].reshape(ND, 1, FD), (ND, 128, FD))
        # lane C
        gtsepb = np.zeros((NCP, 2, 2048), np.float32)
        if NC:
            gtsepb[:NC] = gtdupc[ND:].transpose(0, 2, 1)
        pcolc = np.zeros((NCP, 2, 128, 8), np.float32)
        if NC:
            pcolc[:NC] = (-predc[ND:]).reshape(NC, 8, 128, 2).transpose(0, 3, 2, 1)
        # stationary tiles
        statp = np.zeros((128, NCP, 2, 8, 8), dtype=np.float32)
        stato = np.zeros((128, NCP, 8), dtype=np.float32)
        if NC:
            pblk = predc[ND:].reshape(NC, 8, 128, 2).transpose(2, 0, 3, 1)
            for bi in range(NC):
                statp[:, bi, :, :, bi] = -2.0 * pblk[:, bi, :, :]
                stato[:, bi, bi] = -1.0
        statp = statp.reshape(128, NCP * 16 * 8)
        stato = stato.reshape(128, NCP * 8)
        in_maps.append(
            {
                "gtw": gtdup,
                "prep": prepc,
                "gtsepb": _to_bf16(gtsepb),
                "pcolc": pcolc,
                "statp": _to_bf16(statp),
                "stato": _to_bf16(stato),
            }
        )
    return in_maps


def _to_bf16(a: np.ndarray) -> np.ndarray:
    import ml_dtypes

    return a.astype(ml_dtypes.bfloat16)


def _finish(results, pred: np.ndarray, gt: np.ndarray) -> np.float32:
    pred = np.asarray(pred, dtype=np.float64)
    gt = np.asarray(gt, dtype=np.float64)
    mins = []
    for core in range(NCORES):
        sl = slice(core * BL, (core + 1) * BL)
        # lane D
        acc = np.asarray(results[core]["acc"], dtype=np.float32)  # [128, ND*8]
        acc = acc.reshape(128, ND, 8)  # [i_local, b, qi]
        dis = acc.transpose(1, 2, 0).reshape(ND, PNUM) / (2.0 * PNUM)
        mins.append(dis.min(axis=1))
        # lane C: 2P*dis = qc + psum  (psum = -2corr - sum rsq)
        accc = np.asarray(results[core]["accc"], dtype=np.float64)[:NC]  # [NC,1024]
        pc = pred[sl][ND:]
        gc = gt[sl][ND:]
        qc = (pc * pc).sum(axis=(1, 2)) + (gc * gc).sum(axis=(1, 2))  # [NC]
        disc = (qc[:, None] + accc) / (2.0 * PNUM)
        mins.append(disc.min(axis=1).astype(np.float32))
    return np.asarray(np.mean(np.concatenate(mins)), dtype=np.float32)


def kernel(pred: np.ndarray, gt: np.ndarray) -> np.ndarray:
    nc = _get_program()
    in_maps = _make_in_maps(pred, gt)
    res = bass_utils.run_bass_kernel_spmd(nc, in_maps, list(range(NCORES)))
    return _finish(res.results, pred, gt)


# Exposed for test.py: run with tracing and return (value, BassKernelResults)
def kernel_traced(pred: np.ndarray, gt: np.ndarray, **kw):
    nc = _get_program()
    in_maps = _make_in_maps(pred, gt)
    res = bass_utils.run_bass_kernel_spmd(nc, in_maps, list(range(NCORES)), **kw)
    return _finish(res.results, pred, gt), res


# revision 16
# speedup vs baseline: 1.1625x; 1.1625x over previous
"""PolyMatchingLoss Trainium2 kernel.

Reference computation (B=128, P=1024, C=2):
    dis[b, i] = mean_j sum_c smooth_l1(pred[b,j,c] - gt[b,(i+j)%P,c])
    out = mean_b min_i dis[b, i]

Strategy: two concurrent per-core lanes over the 16 local batches.

  Lane D (DVE, ND batches): two paged-scan custom DVE instructions per
    batch.  Each covers [128 shifts, 4 pages, 2048 (j,c)]: page s is the
    window slice of shift-block qi=4h+s (free-offset stride 256 into the
    same SBUF window tile), in1 is pred broadcast along the page dim
    (stride 0).  The body computes 2*smooth_l1 = m*(2t-m), t=|d|,
    m=min(t,1), wrapped in scan(ADD, .): the out tile holds the running
    prefix sum, and the page-end columns (strided [128,4] copy into acc)
    are cumulative page sums that the host differences.  One instruction
    per 4 shift-blocks amortizes the ~350-cycle DVE per-instruction
    overhead.  The gt operand uses the staircase identity
    W[x, y] = gtflat2[2x+y]; pred is host-replicated to 128 partitions.

  Lane C (ACT+PE, NC batches): uses 2f(d) = d^2 - relu(|d|-1)^2.
    Layout: partition = point-in-block u, free = shift i.
    - Sum_j d^2 = (Sum p^2 + Sum g^2) - 2 corr[i]; the constant is added
      on the host, corr[i] is computed by TensorE as 8 accumulating
      matmuls per c whose stationary operand is the -2*pred block column
      and whose moving operand is the SAME gt window tile the ACT passes
      read.
    - The correction Sum_j relu(|d|-1)^2 runs on ScalarE in 3 passes
      (Abs with per-partition -pred bias; one big Relu(x-1); one big
      Square) and is reduced over partitions by TensorE with a -1s
      stationary column.
    - All matmuls of all NC batches accumulate into one stacked PSUM
      pair [8, 512]x2 via one-hot stationary columns (col = local batch
      index), so PSUM is copied out exactly once per rep.

  min over shifts + mean over batch on host (tiny).
"""

from operator import add as _operator_add

import numpy as np

from concourse import mybir
from concourse import bass, bass_utils
from concourse.tile import TileContext
import concourse.dve_ops as _dve_ops
from concourse.dve_ops import DveOp
from concourse.dve_spec import Spec, Src0, Src1, Zero, One, maxx, minn, Bin, scan
from concourse.dve_uop import AluOp

# ---------------------------------------------------------------------------
# Workaround: this toolchain's walrus allows at most ONE sync wait per
# instruction; Tile emits 2+.  Split extras onto EventSemaphore carrier
# instructions inserted just before the offending instruction.
# ---------------------------------------------------------------------------
def _split_multi_waits(nc) -> int:
    n = 0
    for fn in nc.m.functions:
        for bb in fn.blocks:
            out = []
            for inst in bb.instructions:
                si = inst.sync_info
                if si is not None and si.on_wait and len(si.on_wait) > 1:
                    for k, w in enumerate(si.on_wait[:-1]):
                        out.append(
                            mybir.InstEventSemaphore(
                                name=f"{inst.name}_wsplit{k}",
                                opcode="EventSemaphore",
                                engine=inst.engine,
                                ins=[],
                                outs=[],
                                sync_info=mybir.SyncInfo(on_wait=[w], on_update=[]),
                            )
                        )
                        n += 1
                    si.on_wait = [si.on_wait[-1]]
                out.append(inst)
            bb.instructions = out
    return n


B = 128
PNUM = 1024
C = 2
NCORES = 8
BL = B // NCORES  # batches per core
FD = PNUM * C  # 2048 free elements per lane-D tile
WW = FD + 256 * 7  # 3840 lane-D window width
WB = 1024 + 128 * 7  # 1920 lane-C per-coordinate window width

ND = 12  # lane-D (DVE) batches per core
NC = BL - ND  # lane-C (ACT+PE) batches per core


# --------------------------------------------------------------------------
# Custom DVE op: out = m*(2t - m) with t=|in0-in1|, m=min(t,1)  (= 2*huber)
#                accum_out = sum over free axis
# --------------------------------------------------------------------------
def _huber_ref(in0, in1, s0, s1, imm2):
    dd = in0.astype(np.float32) - in1.astype(np.float32)
    tt = np.abs(dd)
    mm = np.minimum(tt, 1.0)
    bb = (mm * (2.0 * tt - mm)).astype(np.float32)
    return bb, bb.reshape(bb.shape[0], -1).sum(axis=-1, keepdims=True)


def _make_huber_op() -> DveOp:
    d = Src0 - Src1
    nd = Src1 - Src0
    t = maxx(d, nd)
    m = minn(t, One)
    v = t - m
    w = t + v
    body = m * w
    return DveOp(
        "TENSOR_HUBER2_REDUCE",
        Spec(body=body, accum=_operator_add, accum_init=Zero, reference=_huber_ref),
        subdim=False,
        uops_sha={"v3": "e8f6160a1f1db788", "v4": "8b26f7daea78cb80"},
    )


def _register_op(op: DveOp) -> None:
    if op.name in _dve_ops._SUB_OPCODE_FOR_NAME:
        return
    _dve_ops.OPS.append(op)
    _dve_ops._SUB_OPCODE_FOR_NAME[op.name] = (
        _dve_ops._CUSTOM_DVE_ROW_BASE + len(_dve_ops.OPS) - 1
    )
    _dve_ops.CUSTOM_DVE_SPECS[op.name] = op.spec
    assert _dve_ops._SUB_OPCODE_FOR_NAME[op.name] < 0x20


HUBER_OP = _make_huber_op()
_register_op(HUBER_OP)


# Paged-scan variant: body = running prefix sum of 2*huber along the free
# stream of a [128, S, 2048] instruction (S shift-block pages).  The page-end
# columns hold cumulative sums; the host differences them.  One instruction
# covers S shift-blocks, amortizing the ~350-cycle per-instruction overhead.
def _huber_scan_ref(in0, in1, s0, s1, imm2):
    a = in0.astype(np.float32)
    b = np.broadcast_to(in1, in0.shape).astype(np.float32)
    t = np.abs(a - b)
    m = np.minimum(t, 1.0)
    h = m * (2.0 * t - m)
    P = h.shape[0]
    return np.cumsum(h.reshape(P, -1), axis=1).reshape(h.shape).astype(np.float32)


def _make_huber_scan_op() -> DveOp:
    t = Bin(AluOp.ABSOLUTE_DIFF, Src0, Src1)
    m = minn(t, One)
    h = m * (t + (t - m))
    return DveOp(
        "TENSOR_HUBER2_SCAN",
        Spec(body=scan(AluOp.ADD, h), reference=_huber_scan_ref),
        subdim=False,
        uops_sha={"v3": "e8ebb1af571f5afc", "v4": "2f6df639b37b94af"},
    )


HUBER_SCAN_OP = _make_huber_scan_op()
_register_op(HUBER_SCAN_OP)


# --------------------------------------------------------------------------
# Bass program (SPMD, one program for all 8 cores)
# --------------------------------------------------------------------------
_dt = mybir.dt
_program_cache = {}


def _build_program(reps: int = 1):
    nc = bass.Bass()
    AF = mybir.ActivationFunctionType

    NDP, NCP = max(ND, 1), max(NC, 1)
    # lane D inputs
    gtw = nc.declare_dram_parameter("gtw", [NDP, 2 * FD], _dt.float32, isOutput=False)
    prep = nc.declare_dram_parameter(
        "prep", [NDP, 128, FD], _dt.float32, isOutput=False
    )
    # lane C inputs
    gtsepb = nc.declare_dram_parameter(
        "gtsepb", [NCP, 2, 2048], _dt.bfloat16, isOutput=False
    )
    pcolc = nc.declare_dram_parameter(
        "pcolc", [NCP, 2, 128, 8], _dt.float32, isOutput=False
    )
    statp = nc.declare_dram_parameter(
        "statp", [128, NCP * 16 * 8], _dt.bfloat16, isOutput=False
    )
    stato = nc.declare_dram_parameter(
        "stato", [128, NCP * 8], _dt.bfloat16, isOutput=False
    )
    # outputs
    acc_out = nc.declare_dram_parameter(
        "acc", [128, NDP * 8], _dt.float32, isOutput=True
    )
    accc_out = nc.declare_dram_parameter("accc", [8, 1024], _dt.float32, isOutput=True)

    with TileContext(nc) as tc:
        with (
            tc.tile_pool(name="w", bufs=2) as wpool,
            tc.tile_pool(name="p", bufs=2) as ppool,
            tc.tile_pool(name="s", bufs=1) as spool,
            tc.tile_pool(name="a", bufs=1) as apool,
            tc.tile_pool(name="wb", bufs=2) as wbpool,
            tc.tile_pool(name="pc", bufs=2) as pcpool,
            tc.tile_pool(name="act", bufs=2) as actpool,
            tc.tile_pool(name="st", bufs=1) as stpool,
            tc.tile_pool(name="ac2", bufs=2) as ac2pool,
            tc.tile_pool(name="ps", bufs=2, space="PSUM") as pspool,
        ):
            acc = apool.tile([128, max(ND, 1) * 8], _dt.float32)
            nc.vector.memset(acc[:], 0.0)
            statpt = stpool.tile([128, max(NC, 1) * 16 * 8], _dt.bfloat16, tag="statpt")
            nc.sync.dma_start(out=statpt[:], in_=statp[:])
            statot = stpool.tile([128, max(NC, 1) * 8], _dt.bfloat16, tag="statot")
            nc.sync.dma_start(out=statot[:], in_=stato[:])
            onen = stpool.tile([128, 1], _dt.float32, tag="onen")
            nc.vector.memset(onen[:], -1.0)

            for _rep in range(reps):
                if NC > 0:
                    psA = pspool.tile([8, 512], _dt.float32, tag="psA")
                    psB = pspool.tile([8, 512], _dt.float32, tag="psB")

                # ---------------- lane C (ACT + PE) ----------------
                # start/stop are per PSUM bank: the first matmul into EACH
                # of psA/psB must carry start=True (clears the bank), else
                # a second execution of the NEFF accumulates onto stale
                # PSUM contents.
                nmm = 0  # per-bank matmul counter (A and B advance together)
                NMM_TOTAL = NC * 2 * 8 * 2
                for bi in range(NC):
                    wb = wbpool.tile([128, 2 * WB], _dt.bfloat16)
                    for c in (0, 1):
                        # staircase: row u = gtsepb[bi, c, u : u + WB]
                        nc.scalar.dma_start(
                            out=wb[:, c * WB : (c + 1) * WB],
                            in_=bass.AP(
                                gtsepb, (bi * 2 + c) * 2048, [[1, 128], [1, WB]]
                            ),
                        )
                    pcol = pcpool.tile([128, 16], _dt.float32)
                    for c in (0, 1):
                        nc.scalar.dma_start(
                            out=pcol[:, c * 8 : (c + 1) * 8], in_=pcolc[bi, c]
                        )
                    for c in (0, 1):
                        # corr matmuls: -2*corr into psum rows [bi]
                        for q in range(8):
                            scol = ((bi * 2 + c) * 8 + q) * 8
                            for h, ps in ((0, psA), (1, psB)):
                                nc.tensor.matmul(
                                    ps[:, :],
                                    statpt[:, scol : scol + 8],
                                    wb[:, c * WB + 128 * q + 512 * h :][:, 0:512],
                                    start=(nmm == 0),
                                    stop=(nmm == NMM_TOTAL - 1),
                                )
                            nmm += 1
                        # ACT passes
                        tt = actpool.tile([128, 8192], _dt.bfloat16, tag="tt")
                        for q in range(8):
                            nc.scalar.activation(
                                tt[:, 1024 * q : 1024 * (q + 1)],
                                wb[:, c * WB + 128 * q :][:, 0:1024],
                                AF.Abs,
                                bias=pcol[:, c * 8 + q : c * 8 + q + 1],
                            )
                        rr = actpool.tile([128, 8192], _dt.bfloat16, tag="rr")
                        nc.scalar.activation(rr[:], tt[:], AF.Relu, bias=onen[:, 0:1])
                        r2 = actpool.tile([128, 8192], _dt.bfloat16, tag="r2")
                        nc.scalar.activation(r2[:], rr[:], AF.Square)
                        # reduction matmuls: -sum_u r2 into psum rows [bi]
                        for q in range(8):
                            for h, ps in ((0, psA), (1, psB)):
                                nc.tensor.matmul(
                                    ps[:, :],
                                    statot[:, bi * 8 : bi * 8 + 8],
                                    r2[:, 1024 * q + 512 * h :][:, 0:512],
                                    start=(nmm == 0),
                                    stop=(nmm == NMM_TOTAL - 1),
                                )
                            nmm += 1

                # ---------------- lane D (DVE) ----------------
                for b in range(ND):
                    w = wpool.tile([128, WW], _dt.float32)
                    # staircase window: row x = gtflat2[b, 2x : 2x + WW]
                    nc.sync.dma_start(
                        out=w[:], in_=bass.AP(gtw, b * 2 * FD, [[2, 128], [1, WW]])
                    )
                    p = ppool.tile([128, FD], _dt.float32)
                    nc.sync.dma_start(out=p[:], in_=prep[b])
                    p3 = p[:].unsqueeze(1).broadcast_to([128, 4, FD])
                    for h in (0, 1):
                        scr = spool.tile([128, 4 * FD], _dt.float32)
                        wap = w[:]
                        in0 = bass.AP(
                            wap.tensor, 1024 * h, [[WW, 128], [256, 4], [1, FD]]
                        )
                        sap = scr[:]
                        out3 = bass.AP(
                            sap.tensor, 0, [[4 * FD, 128], [FD, 4], [1, FD]]
                        )
                        nc.vector._custom_dve(
                            HUBER_SCAN_OP, out=out3, in0=in0, in1=p3
                        )
                        # page-end columns = cumulative sums through each page
                        col = b * 8 + 4 * h
                        nc.vector.tensor_copy(
                            acc[:, col : col + 4],
                            bass.AP(sap.tensor, FD - 1, [[4 * FD, 128], [FD, 4]]),
                        )

                # psum -> sbuf -> dram for lane C (on ScalarE: the DVE is
                # the binding lane, keep it free of the copies)
                if NC > 0:
                    accc = ac2pool.tile([8, 1024], _dt.float32)
                    nc.scalar.copy(accc[:, 0:512], psA[:])
                    nc.scalar.copy(accc[:, 512:1024], psB[:])
                    nc.scalar.dma_start(out=accc_out[:], in_=accc[:])

            nc.sync.dma_start(out=acc_out[:], in_=acc[:])
    _split_multi_waits(nc)
    # Raw Bass (unlike Bacc.compile) never runs this pass; without it the
    # custom-DVE InstISA subclasses serialize with empty .instr bytes and
    # walrus fails with "ISA wrong length".
    mybir.codegen_inst_isa_subclasses(nc)
    return nc


def _get_program():
    if "nc" not in _program_cache:
        _program_cache["nc"] = _build_program()
    return _program_cache["nc"]


# --------------------------------------------------------------------------
# Host wrapper
# --------------------------------------------------------------------------
def _make_in_maps(pred: np.ndarray, gt: np.ndarray):
    pred = np.ascontiguousarray(pred, dtype=np.float32)
    gt = np.ascontiguousarray(gt, dtype=np.float32)
    in_maps = []
    for core in range(NCORES):
        sl = slice(core * BL, (core + 1) * BL)
        gtc = gt[sl]  # [BL, P, C]
        predc = pred[sl]  # [BL, P, C]
        gtdupc = np.concatenate([gtc, gtc], axis=1)  # [BL, 2P, C]
        # lane D
        NDP, NCP = max(ND, 1), max(NC, 1)
        gtdup = np.zeros((NDP, 2 * FD), np.float32)
        gtdup[:ND] = gtdupc[:ND].reshape(ND, 2 * FD)
        prepc = np.zeros((NDP, 128, FD), np.float32)
        prepc[:ND] = np.broadcast_to(predc[:ND].reshape(ND, 1, FD), (ND, 128, FD))
        # lane C
        gtsepb = np.zeros((NCP, 2, 2048), np.float32)
        if NC:
            gtsepb[:NC] = gtdupc[ND:].transpose(0, 2, 1)
        pcolc = np.zeros((NCP, 2, 128, 8), np.float32)
        if NC:
            pcolc[:NC] = (-predc[ND:]).reshape(NC, 8, 128, 2).transpose(0, 3, 2, 1)
        # stationary tiles
        statp = np.zeros((128, NCP, 2, 8, 8), dtype=np.float32)
        stato = np.zeros((128, NCP, 8), dtype=np.float32)
        if NC:
            pblk = predc[ND:].reshape(NC, 8, 128, 2).transpose(2, 0, 3, 1)
            for bi in range(NC):
                statp[:, bi, :, :, bi] = -2.0 * pblk[:, bi, :, :]
                stato[:, bi, bi] = -1.0
        statp = statp.reshape(128, NCP * 16 * 8)
        stato = stato.reshape(128, NCP * 8)
        in_maps.append(
            {
                "gtw": gtdup,
                "prep": prepc,
                "gtsepb": _to_bf16(gtsepb),
                "pcolc": pcolc,
                "statp": _to_bf16(statp),
                "stato": _to_bf16(stato),
            }
        )
    return in_maps


def _to_bf16(a: np.ndarray) -> np.ndarray:
    import ml_dtypes

    return a.astype(ml_dtypes.bfloat16)


def _finish(results, pred: np.ndarray, gt: np.ndarray) -> np.float32:
    pred = np.asarray(pred, dtype=np.float64)
    gt = np.asarray(gt, dtype=np.float64)
    mins = []
    for core in range(NCORES):
        sl = slice(core * BL, (core + 1) * BL)
        # lane D
        acc = np.asarray(results[core]["acc"], dtype=np.float64)  # [128, ND*8]
        acc = acc.reshape(128, ND, 2, 4)  # [i_local, b, half, page(cumsum)]
        acc = np.diff(acc, axis=3, prepend=0.0).reshape(128, ND, 8)
        dis = acc.transpose(1, 2, 0).reshape(ND, PNUM) / (2.0 * PNUM)
        mins.append(dis.min(axis=1))
        # lane C: 2P*dis = qc + psum  (psum = -2corr - sum rsq)
        accc = np.asarray(results[core]["accc"], dtype=np.float64)[:NC]  # [NC,1024]
        pc = pred[sl][ND:]
        gc = gt[sl][ND:]
        qc = (pc * pc).sum(axis=(1, 2)) + (gc * gc).sum(axis=(1, 2))  # [NC]
        disc = (qc[:, None] + accc) / (2.0 * PNUM)
        mins.append(disc.min(axis=1).astype(np.float32))
    return np.asarray(np.mean(np.concatenate(mins)), dtype=np.float32)


def kernel(pred: np.ndarray, gt: np.ndarray) -> np.ndarray:
    nc = _get_program()
    in_maps = _make_in_maps(pred, gt)
    res = bass_utils.run_bass_kernel_spmd(nc, in_maps, list(range(NCORES)))
    return _finish(res.results, pred, gt)


# Exposed for test.py: run with tracing and return (value, BassKernelResults)
def kernel_traced(pred: np.ndarray, gt: np.ndarray, **kw):
    nc = _get_program()
    in_maps = _make_in_maps(pred, gt)
    res = bass_utils.run_bass_kernel_spmd(nc, in_maps, list(range(NCORES)), **kw)
    return _finish(res.results, pred, gt), res
